# revision 57
# baseline (speedup 1.0000x reference)
"""Trainium2 Bass kernel for nn_AdvancedMemorySystem (retrieval_knn).

out = concat([softmax(x @ W_epi.T + b_epi) @ epi_mem, x]) @ W_cons.T + b_cons
(The semantic-memory branch of the reference is dead code — it never reaches
the output — so it is not computed.)

Distribution (8 NeuronCores): episodic bank sharded along E (6250 rows/core,
padded to 6272 = 49*128). Each core computes partial exp-logits, the partial
softmax denominator z and the partial weighted sum v = exp(L) @ mem for all
2048 tokens. v (with z embedded as column 1024, bf16) is ReduceScattered per
512-token superblock, overlapped with the next superblock's compute; each
core then normalizes and applies the consolidator to its 4x64-token slice.
Softmax max-subtraction is skipped: logits are O(1) by construction (Xavier
weights, unit-normal x), so exp is safe in fp32 and z ~ 5e4.
"""

import sys
from contextlib import ExitStack

import numpy as np
import ml_dtypes

import concourse.bass as bass
import concourse.bacc as bacc
import concourse.mybir as mybir
import concourse.tile as tile
from concourse import bass_utils
from concourse.masks import make_identity

P = 128
H = 1024          # hidden dim
NTOK = 2048       # B*T = 4*512
E = 50000         # episodic rows (real)
ER = E // 8       # 6250 real rows per core
ELOC = 6272       # padded rows per core (49 * 128)
EM = ELOC // P    # 49 e-tiles
KH = H // P       # 8 contraction tiles over H
SBW = 1024        # token superblock width (compute granularity)
NSB = NTOK // SBW # 2 superblocks
RSW = 512         # ReduceScatter chunk width (tokens)
NRS = NTOK // RSW # 4 RS chunks
CHK = RSW // 8    # 64 tokens per core per RS chunk
TSLICE = NRS * CHK  # 256 tokens per core total
VW = H + 2        # v row width: 1024 v + 1 z + 1 pad
N_CORES = 8
BF16 = mybir.dt.bfloat16
F32 = mybir.dt.float32
F8 = mybir.dt.float8e4
nbf16 = ml_dtypes.bfloat16
nf8 = mybir.dt.np(F8)
WT_SCALE = 64.0    # host multiplies wt by this; exp() divides it back out
MEM_SCALE = 16.0   # host multiplies mem by this; z column is scaled to match
DR = mybir.MatmulPerfMode.DoubleRow


def _build():
    nc = bacc.Bacc("TRN2", target_bir_lowering=False, debug=False,
                   num_devices=N_CORES)
    xt_d = nc.declare_dram_parameter("xt", [H, NTOK], F8, isOutput=False)
    wt_d = nc.declare_dram_parameter("wt", [H, ELOC], F8, isOutput=False)
    mem_d = nc.declare_dram_parameter("mem", [ELOC, H], F8, isOutput=False)
    bep_d = nc.declare_dram_parameter("bep", [P, EM], F32, isOutput=False)
    mkp_d = nc.declare_dram_parameter("mkp", [P, EM], F32, isOutput=False)
    xc_d = nc.declare_dram_parameter("xc", [H, TSLICE], F32, isOutput=False)
    wc1_d = nc.declare_dram_parameter("wc1", [H, H], BF16, isOutput=False)
    wc2_d = nc.declare_dram_parameter("wc2", [H + 1, H], F32, isOutput=False)
    out_d = nc.declare_dram_parameter("out", [TSLICE, H], F32, isOutput=True)

    rg = [list(range(N_CORES))]
    EXP = mybir.ActivationFunctionType.Exp
    RS = "ReduceScatter"
    ADD = mybir.AluOpType.add

    with tile.TileContext(nc) as tc, ExitStack() as octx:
        dram = octx.enter_context(tc.tile_pool(name="dram", bufs=1, space="DRAM"))
        v_in = [dram.tile([RSW, VW], BF16, name=f"v_in{s}", tag=f"v_in{s}")
                for s in range(NRS)]
        v_out = [dram.tile([CHK, VW], BF16, name=f"v_out{s}", tag=f"v_out{s}")
                 for s in range(NRS)]

        cst = octx.enter_context(tc.tile_pool(name="cst", bufs=1))
        bep_sb = cst.tile([P, EM], F32)
        nc.sync.dma_start(bep_sb[:], bep_d[:, :])
        mkp_sb = cst.tile([P, EM], F32)
        nc.sync.dma_start(mkp_sb[:], mkp_d[:, :])
        # z column comes out of S.T @ (MEM_SCALE * ones) so that the stored
        # z matches the MEM_SCALE-scaled v and the scale cancels in v/z.
        ones_col = cst.tile([P, 1], F32)
        nc.vector.memset(ones_col[:], MEM_SCALE)

        with ExitStack() as ctx:
            memp = ctx.enter_context(tc.tile_pool(name="memp", bufs=1))
            mem_sb = memp.tile([P, EM, H], F8)
            mem_loaded = False

            ptp = ctx.enter_context(tc.tile_pool(name="ptp", bufs=1))
            xtp = ctx.enter_context(tc.tile_pool(name="xtp", bufs=2))
            wtp = ctx.enter_context(tc.tile_pool(name="wtp", bufs=3))
            vzp = ctx.enter_context(tc.tile_pool(name="vzp", bufs=3))
            sp = ctx.enter_context(tc.tile_pool(name="sp", bufs=2))
            ps1p = ctx.enter_context(tc.tile_pool(name="ps1p", bufs=3, space="PSUM"))
            pzp = ctx.enter_context(tc.tile_pool(name="pzp", bufs=1, space="PSUM"))
            ps2p = ctx.enter_context(tc.tile_pool(name="ps2p", bufs=2, space="PSUM"))

            xt_ap = xt_d.ap().rearrange("(k p) t -> p k t", p=P)
            wt_ap = wt_d.ap().rearrange("(k p) e -> p k e", p=P)

            EGW = 8  # e-tiles per wt streaming DMA
            groups = []
            e0 = 0
            while e0 < EM:
                g = min(EGW, EM - e0)
                groups.append((e0, g))
                e0 += g

            for sb in range(NSB):
                t0 = sb * SBW
                xt_sb = xtp.tile([P, KH, SBW], F8, tag="xt")
                nc.sync.dma_start(xt_sb[:], xt_ap[:, :, t0:t0 + SBW])

                pt_sb = ptp.tile([P, EM, SBW], F8, tag="pt")
                s_sb = sp.tile([P, SBW], F32, tag="s")
                tmp_sb = sp.tile([P, SBW], F32, tag="tmp")
                for (eg0, gw) in groups:
                    wt_sb = wtp.tile([P, KH, EGW * P], F8, tag="wt")
                    nc.sync.dma_start(
                        wt_sb[:, :, :gw * P],
                        wt_ap[:, :, eg0 * P:(eg0 + gw) * P])
                    if not mem_loaded:
                        # issue after the first weight group so the data the
                        # PE needs first wins the SDMA bandwidth race
                        nc.gpsimd.dma_start(
                            mem_sb[:],
                            mem_d.ap().rearrange("(em p) h -> p em h", p=P))
                        mem_loaded = True
                    for j in range(gw):
                        em = eg0 + j
                        ps1a = ps1p.tile([P, RSW], F32, tag="ps1", name="ps1a")
                        ps1b = ps1p.tile([P, RSW], F32, tag="ps1", name="ps1b")
                        for k in range(KH // 2):
                            lhsT = wt_sb[:, 2 * k:2 * k + 2, j * P:(j + 1) * P]
                            nc.tensor.matmul(
                                ps1a[:], lhsT,
                                xt_sb[:, 2 * k:2 * k + 2, 0:RSW],
                                start=(k == 0), stop=(k == KH // 2 - 1),
                                perf_mode=DR)
                            nc.tensor.matmul(
                                ps1b[:], lhsT,
                                xt_sb[:, 2 * k:2 * k + 2, RSW:SBW],
                                start=(k == 0), stop=(k == KH // 2 - 1),
                                perf_mode=DR)
                        nc.scalar.activation(pt_sb[:, em, 0:RSW], ps1a[:], EXP,
                                             bias=bep_sb[:, em:em + 1],
                                             scale=1.0 / WT_SCALE)
                        nc.scalar.activation(pt_sb[:, em, RSW:SBW], ps1b[:],
                                             EXP, bias=bep_sb[:, em:em + 1],
                                             scale=1.0 / WT_SCALE)
                        # z partial: S += exp tile (mask the padded tail tile)
                        if em == 0:
                            nc.vector.tensor_copy(s_sb[:], pt_sb[:, 0, :])
                        elif em == EM - 1:
                            nc.vector.tensor_scalar_mul(
                                tmp_sb[:], pt_sb[:, em, :],
                                mkp_sb[:, em:em + 1])
                            nc.vector.tensor_add(s_sb[:], s_sb[:], tmp_sb[:])
                        else:
                            nc.vector.tensor_add(s_sb[:], s_sb[:],
                                                 pt_sb[:, em, :])

                # z column per token tile: z[tm] = S[:, tm-slice].T @ ones
                pz = pzp.tile([P, SBW // P], F32, tag="pz")
                for tm in range(SBW // P):
                    nc.tensor.matmul(pz[:, tm:tm + 1],
                                     s_sb[:, tm * P:(tm + 1) * P],
                                     ones_col[:], start=True, stop=True)

                for tm in range(SBW // P):
                    ps2 = ps2p.tile([P, H], F32, tag="ps2")
                    for ekp in range(EM // 2):
                        lhsT = pt_sb[:, 2 * ekp:2 * ekp + 2, tm * P:(tm + 1) * P]
                        nc.tensor.matmul(ps2[:, 0:512], lhsT,
                                         mem_sb[:, 2 * ekp:2 * ekp + 2, 0:512],
                                         start=(ekp == 0), stop=False,
                                         perf_mode=DR)
                        nc.tensor.matmul(ps2[:, 512:1024], lhsT,
                                         mem_sb[:, 2 * ekp:2 * ekp + 2, 512:1024],
                                         start=(ekp == 0), stop=False,
                                         perf_mode=DR)
                    lhsT = pt_sb[:, EM - 1, tm * P:(tm + 1) * P]
                    nc.tensor.matmul(ps2[:, 0:512], lhsT,
                                     mem_sb[:, EM - 1, 0:512],
                                     start=False, stop=True)
                    nc.tensor.matmul(ps2[:, 512:1024], lhsT,
                                     mem_sb[:, EM - 1, 512:1024],
                                     start=False, stop=True)
                    vt = vzp.tile([P, VW], BF16, tag="vt")
                    nc.vector.tensor_copy(vt[:, 0:H], ps2[:])
                    nc.vector.tensor_copy(vt[:, H:H + 1], pz[:, tm:tm + 1])
                    nc.vector.memset(vt[:, H + 1:VW], 0.0)
                    r0 = sb * SBW + tm * P
                    nc.gpsimd.dma_start(
                        v_in[r0 // RSW][(r0 % RSW):(r0 % RSW) + P, :], vt[:])
                for h in (2 * sb, 2 * sb + 1):
                    nc.gpsimd.collective_compute(
                        RS, ADD, replica_groups=rg,
                        ins=[v_in[h].opt()], outs=[v_out[h].opt()])

        # ---- consolidator on this core's 4 x 64-token slices ----
        with ExitStack() as ctx:
            cons = ctx.enter_context(tc.tile_pool(name="cons", bufs=1))
            cp2 = ctx.enter_context(tc.tile_pool(name="cp2", bufs=2))
            wcp = ctx.enter_context(tc.tile_pool(name="wcp", bufs=3))
            pstp = ctx.enter_context(tc.tile_pool(name="pstp", bufs=2, space="PSUM"))
            ps3p = ctx.enter_context(tc.tile_pool(name="ps3p", bufs=1, space="PSUM"))

            ident = cons.tile([P, P], BF16)
            make_identity(nc, ident)
            ones_row = cons.tile([1, TSLICE], F32)
            nc.vector.memset(ones_row[:], 1.0)

            consTx = cons.tile([P, KH, TSLICE], F32)
            nc.sync.dma_start(consTx[:],
                              xc_d.ap().rearrange("(k p) t -> p k t", p=P))
            consTe = cons.tile([P, KH, TSLICE], BF16)

            ps3 = [ps3p.tile([P, H], F32, tag=f"ps3_{mt}", name=f"ps3_{mt}")
                   for mt in range(2)]
            # x-part first: no dependency on the collectives, fills the
            # window while the last ReduceScatter drains.
            for kc in range(KH):
                wct = wcp.tile([P, H], F32, tag="wct")
                nc.sync.dma_start(wct[:], wc2_d[kc * P:(kc + 1) * P, :])
                for mt in range(2):
                    lhsT = consTx[:, kc, mt * P:(mt + 1) * P]
                    nc.tensor.matmul(ps3[mt][:, 0:512], lhsT, wct[:, 0:512],
                                     start=(kc == 0), stop=False)
                    nc.tensor.matmul(ps3[mt][:, 512:1024], lhsT,
                                     wct[:, 512:1024],
                                     start=(kc == 0), stop=False)
            wcb = wcp.tile([1, H], F32, tag="wcb")
            nc.sync.dma_start(wcb[:], wc2_d[H:H + 1, :])
            for mt in range(2):
                lhsT = ones_row[:, mt * P:(mt + 1) * P]
                nc.tensor.matmul(ps3[mt][:, 0:512], lhsT, wcb[:, 0:512],
                                 start=False, stop=False)
                nc.tensor.matmul(ps3[mt][:, 512:1024], lhsT, wcb[:, 512:1024],
                                 start=False, stop=False)

            # epi part: v/z per token tile (each token tile = 2 RS chunks);
            # fully per-mt so mt=0 finishes while the last RS drains.
            wc1ts = []
            for kc in range(KH):
                wc1t = cons.tile([P, H], BF16, tag=f"wc1t{kc}",
                                 name=f"wc1t{kc}")
                nc.sync.dma_start(wc1t[:], wc1_d[kc * P:(kc + 1) * P, :])
                wc1ts.append(wc1t)
            for mt in range(2):
                vt2 = cp2.tile([P, VW], BF16, tag="vt2")
                nc.sync.dma_start(vt2[0:CHK, :], v_out[2 * mt][:, :])
                nc.sync.dma_start(vt2[CHK:P, :], v_out[2 * mt + 1][:, :])
                rz = cp2.tile([P, 1], F32, tag="rz")
                nc.vector.reciprocal(rz[:], vt2[:, H:H + 1])
                epi = cp2.tile([P, H], BF16, tag="epi")
                nc.vector.tensor_scalar_mul(epi[:], vt2[:, 0:H], rz[:])
                for hb in range(KH):
                    pst = pstp.tile([P, P], BF16, tag="pst")
                    nc.tensor.transpose(pst[:], epi[:, hb * P:(hb + 1) * P],
                                        ident[:])
                    nc.vector.tensor_copy(consTe[:, hb, mt * P:(mt + 1) * P],
                                          pst[:])
                for kc in range(KH):
                    lhsT = consTe[:, kc, mt * P:(mt + 1) * P]
                    nc.tensor.matmul(ps3[mt][:, 0:512], lhsT,
                                     wc1ts[kc][:, 0:512],
                                     start=False, stop=(kc == KH - 1))
                    nc.tensor.matmul(ps3[mt][:, 512:1024], lhsT,
                                     wc1ts[kc][:, 512:1024],
                                     start=False, stop=(kc == KH - 1))
                outt = cp2.tile([P, H], F32, tag="outt")
                nc.vector.tensor_copy(outt[:], ps3[mt][:])
                nc.gpsimd.dma_start(out_d[mt * P:(mt + 1) * P, :], outt[:])

    nc.finalize()
    return nc


_NC = None


def _get_nc():
    global _NC
    if _NC is None:
        _NC = _build()
    return _NC


def _core_token_idx(c):
    """Global token indices owned by core c, in device output row order."""
    idx = []
    for h in range(NRS):
        start = h * RSW + c * CHK
        idx.extend(range(start, start + CHK))
    return np.array(idx)


def _prep_inputs(x, W_epi, b_epi, epi_mem, W_cons, b_cons):
    xf = np.ascontiguousarray(x.reshape(NTOK, H), dtype=np.float32)
    xT = np.ascontiguousarray(xf.T)                      # [H, NTOK] f32
    xt_f8 = xT.astype(nf8)
    wcT = np.asarray(W_cons, np.float32).T               # [2H, H]
    wc1 = np.ascontiguousarray(wcT[:H]).astype(nbf16)    # epi half, bf16
    wc2 = np.empty((H + 1, H), np.float32)               # x half + bias row
    wc2[:H] = wcT[H:]
    wc2[H] = np.asarray(b_cons, np.float32)
    in_maps = []
    for c in range(N_CORES):
        rows = slice(c * ER, (c + 1) * ER)
        wt = np.zeros((H, ELOC), nf8)
        wt[:, :ER] = (WT_SCALE *
                      np.asarray(W_epi[rows], np.float32).T).astype(nf8)
        mem = np.zeros((ELOC, H), nf8)
        mem[:ER] = (MEM_SCALE *
                    np.asarray(epi_mem[rows], np.float32)).astype(nf8)
        be = np.zeros((ELOC,), np.float32)
        be[:ER] = np.asarray(b_epi[rows], np.float32)
        bep = np.ascontiguousarray(be.reshape(EM, P).T)  # [P, EM]
        mk = np.zeros((ELOC,), np.float32)
        mk[:ER] = 1.0
        mkp = np.ascontiguousarray(mk.reshape(EM, P).T)
        xc = np.ascontiguousarray(xT[:, _core_token_idx(c)])
        in_maps.append({
            "xt": xt_f8, "wt": wt, "mem": mem, "bep": bep, "mkp": mkp,
            "xc": xc, "wc1": wc1, "wc2": wc2,
        })
    return in_maps


def run(x, W_epi, b_epi, epi_mem, W_cons, b_cons, trace=False, **spmd_kwargs):
    nc = _get_nc()
    in_maps = _prep_inputs(x, W_epi, b_epi, epi_mem, W_cons, b_cons)
    res = bass_utils.run_bass_kernel_spmd(
        nc, in_maps, core_ids=list(range(N_CORES)), trace=trace, **spmd_kwargs)
    out = np.empty((NTOK, H), np.float32)
    for c in range(N_CORES):
        out[_core_token_idx(c)] = res.results[c]["out"]
    return out.reshape(4, 512, H), res


def kernel(x, W_epi, b_epi, epi_mem, W_sem=None, b_sem=None, sem_mem=None,
           W_cons=None, b_cons=None):
    out, _ = run(x, W_epi, b_epi, epi_mem, W_cons, b_cons)
    return out


# revision 58
# speedup vs baseline: 1.0036x; 1.0036x over previous
"""Trainium2 Bass kernel for nn_AdvancedMemorySystem (retrieval_knn).

out = concat([softmax(x @ W_epi.T + b_epi) @ epi_mem, x]) @ W_cons.T + b_cons
(The semantic-memory branch of the reference is dead code — it never reaches
the output — so it is not computed.)

Distribution (8 NeuronCores): episodic bank sharded along E (6250 rows/core,
padded to 6272 = 49*128). Each core computes partial exp-logits, the partial
softmax denominator z and the partial weighted sum v = exp(L) @ mem for all
2048 tokens. v (with z embedded as column 1024, bf16) is ReduceScattered per
512-token superblock, overlapped with the next superblock's compute; each
core then normalizes and applies the consolidator to its 4x64-token slice.
Softmax max-subtraction is skipped: logits are O(1) by construction (Xavier
weights, unit-normal x), so exp is safe in fp32 and z ~ 5e4.
"""

import sys
from contextlib import ExitStack

import numpy as np
import ml_dtypes

import concourse.bass as bass
import concourse.bacc as bacc
import concourse.mybir as mybir
import concourse.tile as tile
from concourse import bass_utils
from concourse.masks import make_identity

P = 128
H = 1024          # hidden dim
NTOK = 2048       # B*T = 4*512
E = 50000         # episodic rows (real)
ER = E // 8       # 6250 real rows per core
ELOC = 6272       # padded rows per core (49 * 128)
EM = ELOC // P    # 49 e-tiles
KH = H // P       # 8 contraction tiles over H
SBW = 1024        # token superblock width (compute granularity)
NSB = NTOK // SBW # 2 superblocks
RSW = 512         # ReduceScatter chunk width (tokens)
NRS = NTOK // RSW # 4 RS chunks
CHK = RSW // 8    # 64 tokens per core per RS chunk
TSLICE = NRS * CHK  # 256 tokens per core total
VW = H + 2        # v row width: 1024 v + 1 z + 1 pad
N_CORES = 8
BF16 = mybir.dt.bfloat16
F32 = mybir.dt.float32
F8 = mybir.dt.float8e4
nbf16 = ml_dtypes.bfloat16
nf8 = mybir.dt.np(F8)
WT_SCALE = 64.0    # host multiplies wt by this; exp() divides it back out
MEM_SCALE = 16.0   # host multiplies mem by this; z column is scaled to match
DR = mybir.MatmulPerfMode.DoubleRow


def _build():
    nc = bacc.Bacc("TRN2", target_bir_lowering=False, debug=False,
                   num_devices=N_CORES)
    xt_d = nc.declare_dram_parameter("xt", [H, NTOK], F8, isOutput=False)
    wt_d = nc.declare_dram_parameter("wt", [H, ELOC], F8, isOutput=False)
    mem_d = nc.declare_dram_parameter("mem", [ELOC, H], F8, isOutput=False)
    bep_d = nc.declare_dram_parameter("bep", [P, EM], F32, isOutput=False)
    mkp_d = nc.declare_dram_parameter("mkp", [P, EM], F32, isOutput=False)
    xc_d = nc.declare_dram_parameter("xc", [H, TSLICE], F32, isOutput=False)
    wc1_d = nc.declare_dram_parameter("wc1", [H, H], BF16, isOutput=False)
    wc2_d = nc.declare_dram_parameter("wc2", [H + 1, H], F32, isOutput=False)
    out_d = nc.declare_dram_parameter("out", [TSLICE, H], F32, isOutput=True)

    rg = [list(range(N_CORES))]
    EXP = mybir.ActivationFunctionType.Exp
    RS = "ReduceScatter"
    ADD = mybir.AluOpType.add

    with tile.TileContext(nc) as tc, ExitStack() as octx:
        dram = octx.enter_context(tc.tile_pool(name="dram", bufs=1, space="DRAM"))
        v_in = [dram.tile([RSW, VW], BF16, name=f"v_in{s}", tag=f"v_in{s}")
                for s in range(NRS)]
        v_out = [dram.tile([CHK, VW], BF16, name=f"v_out{s}", tag=f"v_out{s}")
                 for s in range(NRS)]

        cst = octx.enter_context(tc.tile_pool(name="cst", bufs=1))
        bep_sb = cst.tile([P, EM], F32)
        nc.sync.dma_start(bep_sb[:], bep_d[:, :])
        mkp_sb = cst.tile([P, EM], F32)
        nc.sync.dma_start(mkp_sb[:], mkp_d[:, :])
        # z column comes out of S.T @ (MEM_SCALE * ones) so that the stored
        # z matches the MEM_SCALE-scaled v and the scale cancels in v/z.
        ones_col = cst.tile([P, 1], F32)
        nc.vector.memset(ones_col[:], MEM_SCALE)

        with ExitStack() as ctx:
            memp = ctx.enter_context(tc.tile_pool(name="memp", bufs=1))
            mem_sb = memp.tile([P, EM, H], F8)
            nc.gpsimd.dma_start(mem_sb[:],
                                mem_d.ap().rearrange("(em p) h -> p em h", p=P))

            ptp = ctx.enter_context(tc.tile_pool(name="ptp", bufs=1))
            xtp = ctx.enter_context(tc.tile_pool(name="xtp", bufs=2))
            wtp = ctx.enter_context(tc.tile_pool(name="wtp", bufs=3))
            vzp = ctx.enter_context(tc.tile_pool(name="vzp", bufs=3))
            sp = ctx.enter_context(tc.tile_pool(name="sp", bufs=2))
            ps1p = ctx.enter_context(tc.tile_pool(name="ps1p", bufs=3, space="PSUM"))
            pzp = ctx.enter_context(tc.tile_pool(name="pzp", bufs=1, space="PSUM"))
            ps2p = ctx.enter_context(tc.tile_pool(name="ps2p", bufs=2, space="PSUM"))

            xt_ap = xt_d.ap().rearrange("(k p) t -> p k t", p=P)
            wt_ap = wt_d.ap().rearrange("(k p) e -> p k e", p=P)

            EGW = 8  # e-tiles per wt streaming DMA
            groups = []
            e0 = 0
            while e0 < EM:
                g = min(EGW, EM - e0)
                groups.append((e0, g))
                e0 += g

            for sb in range(NSB):
                t0 = sb * SBW
                xt_sb = xtp.tile([P, KH, SBW], F8, tag="xt")
                nc.sync.dma_start(xt_sb[:], xt_ap[:, :, t0:t0 + SBW])

                pt_sb = ptp.tile([P, EM, SBW], F8, tag="pt")
                s_sb = sp.tile([P, SBW], F32, tag="s")
                tmp_sb = sp.tile([P, SBW], F32, tag="tmp")
                for (eg0, gw) in groups:
                    wt_sb = wtp.tile([P, KH, EGW * P], F8, tag="wt")
                    nc.sync.dma_start(
                        wt_sb[:, :, :gw * P],
                        wt_ap[:, :, eg0 * P:(eg0 + gw) * P])
                    for j in range(gw):
                        em = eg0 + j
                        ps1a = ps1p.tile([P, RSW], F32, tag="ps1", name="ps1a")
                        ps1b = ps1p.tile([P, RSW], F32, tag="ps1", name="ps1b")
                        for k in range(KH // 2):
                            lhsT = wt_sb[:, 2 * k:2 * k + 2, j * P:(j + 1) * P]
                            nc.tensor.matmul(
                                ps1a[:], lhsT,
                                xt_sb[:, 2 * k:2 * k + 2, 0:RSW],
                                start=(k == 0), stop=(k == KH // 2 - 1),
                                perf_mode=DR)
                            nc.tensor.matmul(
                                ps1b[:], lhsT,
                                xt_sb[:, 2 * k:2 * k + 2, RSW:SBW],
                                start=(k == 0), stop=(k == KH // 2 - 1),
                                perf_mode=DR)
                        nc.scalar.activation(pt_sb[:, em, 0:RSW], ps1a[:], EXP,
                                             bias=bep_sb[:, em:em + 1],
                                             scale=1.0 / WT_SCALE)
                        nc.scalar.activation(pt_sb[:, em, RSW:SBW], ps1b[:],
                                             EXP, bias=bep_sb[:, em:em + 1],
                                             scale=1.0 / WT_SCALE)
                        # z partial: S += exp tile (mask the padded tail tile)
                        if em == 0:
                            nc.vector.tensor_copy(s_sb[:], pt_sb[:, 0, :])
                        elif em == EM - 1:
                            nc.vector.tensor_scalar_mul(
                                tmp_sb[:], pt_sb[:, em, :],
                                mkp_sb[:, em:em + 1])
                            nc.vector.tensor_add(s_sb[:], s_sb[:], tmp_sb[:])
                        else:
                            nc.vector.tensor_add(s_sb[:], s_sb[:],
                                                 pt_sb[:, em, :])

                # z column per token tile: z[tm] = S[:, tm-slice].T @ ones
                pz = pzp.tile([P, SBW // P], F32, tag="pz")
                for tm in range(SBW // P):
                    nc.tensor.matmul(pz[:, tm:tm + 1],
                                     s_sb[:, tm * P:(tm + 1) * P],
                                     ones_col[:], start=True, stop=True)

                for tm in range(SBW // P):
                    ps2 = ps2p.tile([P, H], F32, tag="ps2")
                    for ekp in range(EM // 2):
                        lhsT = pt_sb[:, 2 * ekp:2 * ekp + 2, tm * P:(tm + 1) * P]
                        nc.tensor.matmul(ps2[:, 0:512], lhsT,
                                         mem_sb[:, 2 * ekp:2 * ekp + 2, 0:512],
                                         start=(ekp == 0), stop=False,
                                         perf_mode=DR)
                        nc.tensor.matmul(ps2[:, 512:1024], lhsT,
                                         mem_sb[:, 2 * ekp:2 * ekp + 2, 512:1024],
                                         start=(ekp == 0), stop=False,
                                         perf_mode=DR)
                    lhsT = pt_sb[:, EM - 1, tm * P:(tm + 1) * P]
                    nc.tensor.matmul(ps2[:, 0:512], lhsT,
                                     mem_sb[:, EM - 1, 0:512],
                                     start=False, stop=True)
                    nc.tensor.matmul(ps2[:, 512:1024], lhsT,
                                     mem_sb[:, EM - 1, 512:1024],
                                     start=False, stop=True)
                    vt = vzp.tile([P, VW], BF16, tag="vt")
                    nc.vector.tensor_copy(vt[:, 0:H], ps2[:])
                    nc.vector.tensor_copy(vt[:, H:H + 1], pz[:, tm:tm + 1])
                    nc.vector.memset(vt[:, H + 1:VW], 0.0)
                    r0 = sb * SBW + tm * P
                    nc.gpsimd.dma_start(
                        v_in[r0 // RSW][(r0 % RSW):(r0 % RSW) + P, :], vt[:])
                for h in (2 * sb, 2 * sb + 1):
                    nc.gpsimd.collective_compute(
                        RS, ADD, replica_groups=rg,
                        ins=[v_in[h].opt()], outs=[v_out[h].opt()])

        # ---- consolidator on this core's 4 x 64-token slices ----
        with ExitStack() as ctx:
            cons = ctx.enter_context(tc.tile_pool(name="cons", bufs=1))
            cp2 = ctx.enter_context(tc.tile_pool(name="cp2", bufs=2))
            wcp = ctx.enter_context(tc.tile_pool(name="wcp", bufs=3))
            pstp = ctx.enter_context(tc.tile_pool(name="pstp", bufs=2, space="PSUM"))
            ps3p = ctx.enter_context(tc.tile_pool(name="ps3p", bufs=1, space="PSUM"))

            ident = cons.tile([P, P], BF16)
            make_identity(nc, ident)
            ones_row = cons.tile([1, TSLICE], F32)
            nc.vector.memset(ones_row[:], 1.0)

            consTx = cons.tile([P, KH, TSLICE], F32)
            nc.sync.dma_start(consTx[:],
                              xc_d.ap().rearrange("(k p) t -> p k t", p=P))
            consTe = cons.tile([P, KH, TSLICE], BF16)

            ps3 = [ps3p.tile([P, H], F32, tag=f"ps3_{mt}", name=f"ps3_{mt}")
                   for mt in range(2)]
            # x-part first: no dependency on the collectives, fills the
            # window while the last ReduceScatter drains.
            for kc in range(KH):
                wct = wcp.tile([P, H], F32, tag="wct")
                nc.sync.dma_start(wct[:], wc2_d[kc * P:(kc + 1) * P, :])
                for mt in range(2):
                    lhsT = consTx[:, kc, mt * P:(mt + 1) * P]
                    nc.tensor.matmul(ps3[mt][:, 0:512], lhsT, wct[:, 0:512],
                                     start=(kc == 0), stop=False)
                    nc.tensor.matmul(ps3[mt][:, 512:1024], lhsT,
                                     wct[:, 512:1024],
                                     start=(kc == 0), stop=False)
            wcb = wcp.tile([1, H], F32, tag="wcb")
            nc.sync.dma_start(wcb[:], wc2_d[H:H + 1, :])
            for mt in range(2):
                lhsT = ones_row[:, mt * P:(mt + 1) * P]
                nc.tensor.matmul(ps3[mt][:, 0:512], lhsT, wcb[:, 0:512],
                                 start=False, stop=False)
                nc.tensor.matmul(ps3[mt][:, 512:1024], lhsT, wcb[:, 512:1024],
                                 start=False, stop=False)

            # epi part: v/z per token tile (each token tile = 2 RS chunks);
            # fully per-mt so mt=0 finishes while the last RS drains.
            wc1ts = []
            for kc in range(KH):
                wc1t = cons.tile([P, H], BF16, tag=f"wc1t{kc}",
                                 name=f"wc1t{kc}")
                nc.sync.dma_start(wc1t[:], wc1_d[kc * P:(kc + 1) * P, :])
                wc1ts.append(wc1t)
            for mt in range(2):
                vt2 = cp2.tile([P, VW], BF16, tag="vt2")
                nc.sync.dma_start(vt2[0:CHK, :], v_out[2 * mt][:, :])
                nc.sync.dma_start(vt2[CHK:P, :], v_out[2 * mt + 1][:, :])
                rz = cp2.tile([P, 1], F32, tag="rz")
                nc.vector.reciprocal(rz[:], vt2[:, H:H + 1])
                epi = cp2.tile([P, H], BF16, tag="epi")
                nc.vector.tensor_scalar_mul(epi[:], vt2[:, 0:H], rz[:])
                for hb in range(KH):
                    pst = pstp.tile([P, P], BF16, tag="pst")
                    nc.tensor.transpose(pst[:], epi[:, hb * P:(hb + 1) * P],
                                        ident[:])
                    nc.vector.tensor_copy(consTe[:, hb, mt * P:(mt + 1) * P],
                                          pst[:])
                for kc in range(KH):
                    lhsT = consTe[:, kc, mt * P:(mt + 1) * P]
                    nc.tensor.matmul(ps3[mt][:, 0:512], lhsT,
                                     wc1ts[kc][:, 0:512],
                                     start=False, stop=(kc == KH - 1))
                    nc.tensor.matmul(ps3[mt][:, 512:1024], lhsT,
                                     wc1ts[kc][:, 512:1024],
                                     start=False, stop=(kc == KH - 1))
                outt = cp2.tile([P, H], F32, tag="outt")
                nc.vector.tensor_copy(outt[:], ps3[mt][:])
                nc.gpsimd.dma_start(out_d[mt * P:(mt + 1) * P, :], outt[:])

    nc.finalize()
    return nc


_NC = None


def _get_nc():
    global _NC
    if _NC is None:
        _NC = _build()
    return _NC


def _core_token_idx(c):
    """Global token indices owned by core c, in device output row order."""
    idx = []
    for h in range(NRS):
        start = h * RSW + c * CHK
        idx.extend(range(start, start + CHK))
    return np.array(idx)


def _prep_inputs(x, W_epi, b_epi, epi_mem, W_cons, b_cons):
    xf = np.ascontiguousarray(x.reshape(NTOK, H), dtype=np.float32)
    xT = np.ascontiguousarray(xf.T)                      # [H, NTOK] f32
    xt_f8 = xT.astype(nf8)
    wcT = np.asarray(W_cons, np.float32).T               # [2H, H]
    wc1 = np.ascontiguousarray(wcT[:H]).astype(nbf16)    # epi half, bf16
    wc2 = np.empty((H + 1, H), np.float32)               # x half + bias row
    wc2[:H] = wcT[H:]
    wc2[H] = np.asarray(b_cons, np.float32)
    in_maps = []
    for c in range(N_CORES):
        rows = slice(c * ER, (c + 1) * ER)
        wt = np.zeros((H, ELOC), nf8)
        wt[:, :ER] = (WT_SCALE *
                      np.asarray(W_epi[rows], np.float32).T).astype(nf8)
        mem = np.zeros((ELOC, H), nf8)
        mem[:ER] = (MEM_SCALE *
                    np.asarray(epi_mem[rows], np.float32)).astype(nf8)
        be = np.zeros((ELOC,), np.float32)
        be[:ER] = np.asarray(b_epi[rows], np.float32)
        bep = np.ascontiguousarray(be.reshape(EM, P).T)  # [P, EM]
        mk = np.zeros((ELOC,), np.float32)
        mk[:ER] = 1.0
        mkp = np.ascontiguousarray(mk.reshape(EM, P).T)
        xc = np.ascontiguousarray(xT[:, _core_token_idx(c)])
        in_maps.append({
            "xt": xt_f8, "wt": wt, "mem": mem, "bep": bep, "mkp": mkp,
            "xc": xc, "wc1": wc1, "wc2": wc2,
        })
    return in_maps


def run(x, W_epi, b_epi, epi_mem, W_cons, b_cons, trace=False, **spmd_kwargs):
    nc = _get_nc()
    in_maps = _prep_inputs(x, W_epi, b_epi, epi_mem, W_cons, b_cons)
    res = bass_utils.run_bass_kernel_spmd(
        nc, in_maps, core_ids=list(range(N_CORES)), trace=trace, **spmd_kwargs)
    out = np.empty((NTOK, H), np.float32)
    for c in range(N_CORES):
        out[_core_token_idx(c)] = res.results[c]["out"]
    return out.reshape(4, 512, H), res


def kernel(x, W_epi, b_epi, epi_mem, W_sem=None, b_sem=None, sem_mem=None,
           W_cons=None, b_cons=None):
    out, _ = run(x, W_epi, b_epi, epi_mem, W_cons, b_cons)
    return out
